# revision 9
# baseline (speedup 1.0000x reference)
"""Multi-head attention (B=4, N=2048, DIM=64, H=8) on 8 TRN2 NeuronCores.

Sharding: head-parallel tensor parallelism. Each core owns one head h:
  - gets x^T (bf16, host-pretransposed, DMA'd twice to duplicate across the
    two PE row-group partition halves), the head's [Wq|Wk] / Wv slices
    (bf16, stacked for both PE row-groups) and an augmented Wproj
    [65 x 65] whose 65th column passes the softmax denominator l through.
  - scores are computed transposed (S^T = k @ q^T) so attn@V can chain
    directly; the denominator arrives via an appended ones-column on V
    (row 64 of the AV output accumulates sum_m exp(s)).
  - exp() is split across TWO engines per chunk: most key-tiles evacuate
    PSUM via ScalarE ACTIVATE(Exp); a tunable subset evacuates via a
    single VectorE tensor_scalar that computes the Schraudolph bit-trick
    exp (int16(s*A+B) reinterpreted as bf16), doubling exp throughput.
    Softmax normalization cancels nearly all of the approximation error.
  - the whole kernel is software-pipelined at key-tile granularity:
    chunk i's scores+exp interleave with chunk i-1's attn@V and with the
    next batch's qkv prep, so the exp engines never sit in the PE
    dependency loop and the PE never idles long enough to re-throttle.
  - projection uses a stationary augmented weight (loaded once):
    yT = wp_aug^T @ oT gives 64 unnormalized output channels plus l in
    row 64. No normalization on device: the host divides by l, sums the
    8 heads, transposes, and adds the bias during unsharding.
"""

import os
import sys

import numpy as np

for _p in ("/opt/trn_rl_repo",):
    if os.path.isdir(_p) and _p not in sys.path:
        sys.path.insert(0, _p)

from contextlib import ExitStack

import ml_dtypes
import concourse.bass as bass
import concourse.tile as tile
from concourse import bacc, mybir
from concourse.bass import ds, ts
from concourse.bass_utils import run_bass_kernel_spmd

B, N, C, H = 4, 2048, 64, 8
SCALE = C ** -0.5
NCORES = 8
P = 128            # SBUF/PSUM partitions
NB = N // P        # 16 token blocks per batch
CH = 1024          # attention column chunk (PSUM tile free size)
NCH = N // CH      # 2
MMF = 512          # max fp32-PSUM moving free dim per matmul
F32 = mybir.dt.float32
BF16 = mybir.dt.bfloat16
I16 = mybir.dt.int16
EXP = mybir.ActivationFunctionType.Exp
MUL = mybir.AluOpType.mult
ADD = mybir.AluOpType.add

# Schraudolph exp constants (bf16 bit-pattern via int16):
#   exp(s*SCALE) ~= bitcast_bf16(int16(s * SEXP_A + SEXP_B))
SEXP_A = 128.0 * SCALE / np.log(2.0)
SEXP_C = 7.4                              # zero-mean correction
SEXP_B = 127.0 * 128.0 - SEXP_C

# Which of the 16 key-tiles per chunk are exp'd on VectorE (Schraudolph)
# instead of ScalarE ACTIVATE. Tunable load-balance knob.
N_DVE_TILES = 5
DVE_TILES = {t for t in range(NB)
             if ((t + 1) * N_DVE_TILES) // NB > (t * N_DVE_TILES) // NB}


def _load_x(nc, pools, x, b):
    """x[b]^T duplicated across both partition halves straight from DRAM."""
    xT = pools["xTp"].tile([P, N], BF16, tag="xT", name=f"xT{b % 2}")
    nc.sync.dma_start(out=xT[0:C, :], in_=x[b])
    nc.sync.dma_start(out=xT[C:P, :], in_=x[b])
    return xT


def _prep_qk_ops(nc, pools, prep):
    """Yield per-step closures computing qT/kT for a prepped batch."""
    ps_sm, wqk_sb = pools["ps_sm"], pools["wqk_sb"]
    xT = prep["xT"]
    qT = pools["qTp"].tile([P, N], BF16, tag="qT")
    kT = pools["kTp"].tile([P, NB, P], BF16, tag="kT")
    prep["qT"], prep["kT"] = qT, kT
    for j in range(N // MMF):
        g = j % 2
        psqk = ps_sm.tile([P, MMF], F32, tag="sm", name=f"psqk{j % 2}")
        yield lambda psqk=psqk, g=g, j=j: nc.tensor.matmul(
            psqk, lhsT=wqk_sb[ds(C * g, C), :],
            rhs=xT[ds(C * g, C), ts(j, MMF)], start=True, stop=True)

        def evac(psqk=psqk, j=j):
            nc.vector.tensor_copy(out=qT[0:C, ts(j, MMF)], in_=psqk[0:C, :])
            kv = kT[:, 4 * j:4 * j + 4, :]
            nc.vector.tensor_copy(out=kv[0:C].rearrange("p a m -> p (a m)"),
                                  in_=psqk[C:P, :])
            # incremental row-group duplication, same engine as the writes
            # above so ordering is free and the first score tiles of the
            # batch can start as soon as j=0 lands
            nc.vector.tensor_copy(out=qT[C:P, ts(j, MMF)],
                                  in_=qT[0:C, ts(j, MMF)])
            nc.vector.tensor_copy(
                out=kT[C:P, 4 * j:4 * j + 4].rearrange("p a m -> p (a m)"),
                in_=kT[0:C, 4 * j:4 * j + 4].rearrange("p a m -> p (a m)"))
        yield evac


def _prep_v_ops(nc, pools, prep):
    """Yield per-step closures computing vaug for a prepped batch."""
    ps_sm, wv_sb = pools["ps_sm"], pools["wv_sb"]
    xT = prep["xT"]
    vaug = pools["vp"].tile([P, NB, C + 1], BF16, tag="vaug")
    prep["vaug"] = vaug
    yield lambda: nc.vector.memset(vaug[:, :, C:C + 1], 1.0)
    for g in range(2):
        psv = ps_sm.tile([P, NB // 2, C], F32, tag="sm", name=f"psv{g}")
        for u in range(NB // 2):
            yield lambda psv=psv, u=u, g=g: nc.tensor.matmul(
                psv[:, u, :], lhsT=xT[ds(C * g, C), ts(2 * u + g, P)],
                rhs=wv_sb[ds(C * g, C), :], start=True, stop=True)
        yield lambda psv=psv, g=g: nc.vector.tensor_copy(
            out=vaug[:, g:NB:2, 0:C], in_=psv)


def _attn_kernel(ctx, tc, y, x, wqk, wv, wp):
    nc = tc.nc
    pools = {}
    consts = ctx.enter_context(tc.tile_pool(name="consts", bufs=1))
    for name, bufs in [("xTp", 2), ("qTp", 2), ("kTp", 2), ("vp", 2),
                       ("pTp", 18), ("oTp", 2), ("yp", 2)]:
        pools[name] = ctx.enter_context(tc.tile_pool(name=name, bufs=bufs))
    pools["ps_s"] = ctx.enter_context(
        tc.tile_pool(name="ps_s", bufs=2, space="PSUM"))
    pools["ps_av"] = ctx.enter_context(
        tc.tile_pool(name="ps_av", bufs=1, space="PSUM"))
    pools["ps_sm"] = ctx.enter_context(
        tc.tile_pool(name="ps_sm", bufs=2, space="PSUM"))

    wqk_sb = consts.tile([P, P], BF16)
    nc.gpsimd.dma_start(out=wqk_sb, in_=wqk)
    wv_sb = consts.tile([P, C], BF16)
    nc.gpsimd.dma_start(out=wv_sb, in_=wv)
    wp_sb = consts.tile([C + 1, C + 1], BF16)
    nc.gpsimd.dma_start(out=wp_sb, in_=wp)
    pools.update(wqk_sb=wqk_sb, wv_sb=wv_sb, wp_sb=wp_sb)

    chunks = [(b, ch) for b in range(B) for ch in range(NCH)]
    preps = {0: {"xT": _load_x(nc, pools, x, 0)}}

    pT_store = {}
    yT_store = {}
    av_store = {}
    for i in range(len(chunks) + 1):
        curr = chunks[i] if i < len(chunks) else None
        prev = chunks[i - 1] if i > 0 else None

        # extra work interleaved into this pipeline stage: x DMA for the
        # next batch at its first chunk, qk/v prep ops spread over the
        # steps (batch 0's prep interleaves into its own first chunk).
        prep_ops = []
        if curr is not None:
            b, ch = curr
            if i == 0:
                prep_ops = list(_prep_qk_ops(nc, pools, preps[0])) + \
                           list(_prep_v_ops(nc, pools, preps[0]))
            if ch == 0 and b + 1 < B:
                preps[b + 1] = {"xT": _load_x(nc, pools, x, b + 1)}
            if ch == NCH - 1 and b + 1 < B:
                prep_ops = list(_prep_qk_ops(nc, pools, preps[b + 1])) + \
                           list(_prep_v_ops(nc, pools, preps[b + 1]))
            prep = preps[b]
            av = pools["ps_av"].tile([C + 1, CH], F32, tag="av")
            av_store[curr] = av
            if ch == 0:
                yT_store[b] = pools["yp"].tile([C + 1, N], F32, tag="yT",
                                               name=f"yT{b % 2}")
        if prev is not None:
            pvaug = preps[prev[0]]["vaug"]
            pav = av_store[prev]

        ops_per_step = -(-len(prep_ops) // (NB // 2))
        for u in range(NB // 2):
            # prep ops first: batch-0 scores depend on them, and the PE
            # queue is in-order
            for op in prep_ops[u * ops_per_step:(u + 1) * ops_per_step]:
                op()
            if curr is not None:
                qT, kT = prep["qT"], prep["kT"]
                # score matmuls for a PAIR of key tiles: tile 2u on PE
                # row-group 0, tile 2u+1 on row-group 1, both reading the
                # SAME query columns (via the duplicated qT halves) so the
                # two row-groups share one rhs stream at full rate.
                sp = [pools["ps_s"].tile([P, CH], F32, tag="s",
                                         name=f"s{g}") for g in range(2)]
                for s in range(CH // MMF):
                    for g in range(2):
                        nc.tensor.matmul(
                            sp[g][:, ts(s, MMF)],
                            lhsT=kT[ds(C * g, C), 2 * u + g, :],
                            rhs=qT[ds(C * g, C), ds(ch * CH + s * MMF, MMF)],
                            start=True, stop=True)
                for g in range(2):
                    t = 2 * u + g
                    pT = pools["pTp"].tile([P, CH], BF16, tag="p",
                                           name=f"pT{t}")
                    if t in DVE_TILES:
                        nc.vector.tensor_scalar(out=pT.bitcast(I16),
                                                in0=sp[g],
                                                scalar1=float(SEXP_A),
                                                scalar2=float(SEXP_B),
                                                op0=MUL, op1=ADD)
                    else:
                        nc.scalar.activation(pT, sp[g], EXP, scale=SCALE)
                    pT_store[(curr, t)] = pT
            if prev is not None:
                for t in (2 * u, 2 * u + 1):
                    ppT = pT_store.pop((prev, t))
                    for s in range(CH // MMF):
                        nc.tensor.matmul(pav[:, ts(s, MMF)],
                                         lhsT=pvaug[:, t, :],
                                         rhs=ppT[:, ts(s, MMF)],
                                         start=(t == 0), stop=(t == NB - 1))

        if prev is not None:
            pb, pch = prev
            oT = pools["oTp"].tile([C + 1, CH], BF16, tag="oT")
            nc.vector.tensor_copy(out=oT, in_=av_store.pop(prev))
            yT_sb = yT_store[pb]
            for s in range(CH // MMF):
                psy = pools["ps_sm"].tile([C + 1, MMF], F32, tag="sm",
                                          name=f"psy{s}")
                nc.tensor.matmul(psy, lhsT=pools["wp_sb"],
                                 rhs=oT[:, ts(s, MMF)], start=True, stop=True)
                nc.vector.tensor_copy(
                    out=yT_sb[:, ds(pch * CH + s * MMF, MMF)], in_=psy)
            if pch == NCH - 1:
                nc.sync.dma_start(out=y[pb], in_=yT_store.pop(pb))


def build_kernel_nc():
    nc = bacc.Bacc("TRN2", target_bir_lowering=False, debug=False,
                   num_devices=NCORES)
    x = nc.dram_tensor("x", [B, C, N], BF16, kind="ExternalInput").ap()
    wqk = nc.dram_tensor("wqk", [P, P], BF16, kind="ExternalInput").ap()
    wv = nc.dram_tensor("wv", [P, C], BF16, kind="ExternalInput").ap()
    wp = nc.dram_tensor("wp", [C + 1, C + 1], BF16, kind="ExternalInput").ap()
    y = nc.dram_tensor("y", [B, C + 1, N], F32, kind="ExternalOutput").ap()
    with tile.TileContext(nc) as tc:
        with ExitStack() as ctx:
            _attn_kernel(ctx, tc, y, x, wqk, wv, wp)
    nc.compile()
    return nc


def make_in_maps(x, Wqkv, Wproj, bproj):
    x = np.asarray(x, dtype=np.float32)
    Wqkv = np.asarray(Wqkv, dtype=np.float32)
    Wproj = np.asarray(Wproj, dtype=np.float32)
    x_bf = np.ascontiguousarray(
        x.transpose(0, 2, 1).astype(ml_dtypes.bfloat16))

    def dup(w):  # stack for the two PE row-groups
        return np.ascontiguousarray(
            np.concatenate([w, w], axis=0).astype(ml_dtypes.bfloat16))

    in_maps = []
    for h in range(NCORES):
        wq = Wqkv[:, 0 * H * C + h * C:0 * H * C + (h + 1) * C]
        wk = Wqkv[:, 1 * H * C + h * C:1 * H * C + (h + 1) * C]
        wv = Wqkv[:, 2 * H * C + h * C:2 * H * C + (h + 1) * C]
        wqk = dup(np.concatenate([wq, wk], axis=1))
        # augmented projection: rows = 64 channels + l, cols = 64 outputs
        # + l passthrough
        wp = np.zeros((C + 1, C + 1), np.float32)
        wp[0:C, 0:C] = Wproj[h * C:(h + 1) * C, :]
        wp[C, C] = 1.0
        wp = np.ascontiguousarray(wp.astype(ml_dtypes.bfloat16))
        in_maps.append({"x": x_bf, "wqk": wqk, "wv": dup(wv), "wp": wp})
    return in_maps


_NC_CACHE = None


def _get_nc():
    global _NC_CACHE
    if _NC_CACHE is None:
        _NC_CACHE = build_kernel_nc()
    return _NC_CACHE


def run(inputs, trace=False, trace_kwargs=None):
    in_maps = make_in_maps(**inputs)
    res = run_bass_kernel_spmd(_get_nc(), in_maps, list(range(NCORES)),
                               trace=trace, **(trace_kwargs or {}))
    y = np.zeros((B, N, C), np.float32)
    for r in res.results:
        yT = r["y"].reshape(B, C + 1, N).astype(np.float32)
        y += (yT[:, 0:C, :] / yT[:, C:C + 1, :]).transpose(0, 2, 1)
    y += np.asarray(inputs["bproj"], np.float32)[None, None, :]
    return y, res


def kernel(x, Wqkv, Wproj, bproj):
    y, _ = run(dict(x=x, Wqkv=Wqkv, Wproj=Wproj, bproj=bproj))
    return y


# revision 10
# speedup vs baseline: 1.2305x; 1.2305x over previous
"""Multi-head attention (B=4, N=2048, DIM=64, H=8) on 8 TRN2 NeuronCores.

Sharding: head-parallel tensor parallelism. Each core owns one head h:
  - gets x^T (bf16, host-pretransposed, DMA'd twice to duplicate across the
    two PE row-group partition halves), the head's [Wq|Wk] / Wv slices
    (bf16, stacked for both PE row-groups) and an augmented Wproj
    [65 x 65] whose 65th column passes the softmax denominator l through.
  - scores are computed transposed (S^T = k @ q^T) so attn@V can chain
    directly; the denominator arrives via an appended ones-column on V
    (row 64 of the AV output accumulates sum_m exp(s)).
  - exp() is split across TWO engines per chunk: most key-tiles evacuate
    PSUM via ScalarE ACTIVATE(Exp); a tunable subset evacuates via a
    single VectorE tensor_scalar that computes the Schraudolph bit-trick
    exp (int16(s*A+B) reinterpreted as bf16), doubling exp throughput.
    Softmax normalization cancels nearly all of the approximation error.
  - the whole kernel is software-pipelined at key-tile granularity:
    chunk i's scores+exp interleave with chunk i-1's attn@V and with the
    next batch's qkv prep, so the exp engines never sit in the PE
    dependency loop and the PE never idles long enough to re-throttle.
  - projection uses a stationary augmented weight (loaded once):
    yT = wp_aug^T @ oT gives 64 unnormalized output channels plus l in
    row 64. No normalization on device: the host divides by l, sums the
    8 heads, transposes, and adds the bias during unsharding.
"""

import os
import sys

import numpy as np

for _p in ("/opt/trn_rl_repo",):
    if os.path.isdir(_p) and _p not in sys.path:
        sys.path.insert(0, _p)

from contextlib import ExitStack

import ml_dtypes
import concourse.bass as bass
import concourse.tile as tile
from concourse import bacc, mybir
from concourse.bass import ds, ts
from concourse.bass_utils import run_bass_kernel_spmd

B, N, C, H = 4, 2048, 64, 8
SCALE = C ** -0.5
NCORES = 8
P = 128            # SBUF/PSUM partitions
NB = N // P        # 16 token blocks per batch
CH = 1024          # attention column chunk (PSUM tile free size)
NCH = N // CH      # 2
MMF = 512          # max fp32-PSUM moving free dim per matmul
F32 = mybir.dt.float32
BF16 = mybir.dt.bfloat16
I16 = mybir.dt.int16
EXP = mybir.ActivationFunctionType.Exp
MUL = mybir.AluOpType.mult
ADD = mybir.AluOpType.add

# Schraudolph exp constants (bf16 bit-pattern via int16):
#   exp(s*SCALE) ~= bitcast_bf16(int16(s * SEXP_A + SEXP_B))
SEXP_A = 128.0 * SCALE / np.log(2.0)
SEXP_C = 7.4                              # zero-mean correction
SEXP_B = 127.0 * 128.0 - SEXP_C

# Which of the 16 key-tiles per chunk are exp'd on VectorE (Schraudolph)
# instead of ScalarE ACTIVATE. Tunable load-balance knob.
N_DVE_TILES = 5
DVE_TILES = {t for t in range(NB)
             if ((t + 1) * N_DVE_TILES) // NB > (t * N_DVE_TILES) // NB}


def _load_x(nc, pools, x, b):
    """x[b]^T duplicated across both partition halves straight from DRAM."""
    xT = pools["xTp"].tile([P, N], BF16, tag="xT", name=f"xT{b % 2}")
    nc.sync.dma_start(out=xT[0:C, :], in_=x[b])
    nc.sync.dma_start(out=xT[C:P, :], in_=x[b])
    return xT


def _prep_qk_ops(nc, pools, prep):
    """Yield per-step closures computing qT/kT for a prepped batch."""
    ps_sm, wqk_sb = pools["ps_sm"], pools["wqk_sb"]
    xT = prep["xT"]
    qT = pools["qTp"].tile([P, N], BF16, tag="qT")
    kT = pools["kTp"].tile([P, NB, P], BF16, tag="kT")
    prep["qT"], prep["kT"] = qT, kT
    for j in range(N // MMF):
        g = j % 2
        psqk = ps_sm.tile([P, MMF], F32, tag="sm", name=f"psqk{j % 2}")
        yield lambda psqk=psqk, g=g, j=j: nc.tensor.matmul(
            psqk, lhsT=wqk_sb[ds(C * g, C), :],
            rhs=xT[ds(C * g, C), ts(j, MMF)], start=True, stop=True)

        def evac(psqk=psqk, j=j):
            nc.vector.tensor_copy(out=qT[0:C, ts(j, MMF)], in_=psqk[0:C, :])
            kv = kT[:, 4 * j:4 * j + 4, :]
            nc.vector.tensor_copy(out=kv[0:C].rearrange("p a m -> p (a m)"),
                                  in_=psqk[C:P, :])
            # incremental row-group duplication, same engine as the writes
            # above so ordering is free and the first score tiles of the
            # batch can start as soon as j=0 lands
            nc.vector.tensor_copy(out=qT[C:P, ts(j, MMF)],
                                  in_=qT[0:C, ts(j, MMF)])
            nc.vector.tensor_copy(
                out=kT[C:P, 4 * j:4 * j + 4].rearrange("p a m -> p (a m)"),
                in_=kT[0:C, 4 * j:4 * j + 4].rearrange("p a m -> p (a m)"))
        yield evac


def _prep_v_ops(nc, pools, prep):
    """Yield per-step closures computing vaug for a prepped batch."""
    ps_sm, wv_sb = pools["ps_sm"], pools["wv_sb"]
    xT = prep["xT"]
    vaug = pools["vp"].tile([P, NB, C + 1], BF16, tag="vaug")
    prep["vaug"] = vaug
    yield lambda: nc.vector.memset(vaug[:, :, C:C + 1], 1.0)
    for g in range(2):
        psv = ps_sm.tile([P, NB // 2, C], F32, tag="sm", name=f"psv{g}")
        for u in range(NB // 2):
            yield lambda psv=psv, u=u, g=g: nc.tensor.matmul(
                psv[:, u, :], lhsT=xT[ds(C * g, C), ts(2 * u + g, P)],
                rhs=wv_sb[ds(C * g, C), :], start=True, stop=True)
        yield lambda psv=psv, g=g: nc.vector.tensor_copy(
            out=vaug[:, g:NB:2, 0:C], in_=psv)


def _attn_kernel(ctx, tc, y, x, wqk, wv, wp):
    nc = tc.nc
    pools = {}
    consts = ctx.enter_context(tc.tile_pool(name="consts", bufs=1))
    for name, bufs in [("xTp", 2), ("qTp", 2), ("kTp", 2), ("vp", 2),
                       ("pTp", 18), ("oTp", 2), ("yp", 2)]:
        pools[name] = ctx.enter_context(tc.tile_pool(name=name, bufs=bufs))
    pools["ps_s"] = ctx.enter_context(
        tc.tile_pool(name="ps_s", bufs=2, space="PSUM"))
    pools["ps_av"] = ctx.enter_context(
        tc.tile_pool(name="ps_av", bufs=1, space="PSUM"))
    pools["ps_sm"] = ctx.enter_context(
        tc.tile_pool(name="ps_sm", bufs=2, space="PSUM"))

    wqk_sb = consts.tile([P, P], BF16)
    nc.gpsimd.dma_start(out=wqk_sb, in_=wqk)
    wv_sb = consts.tile([P, C], BF16)
    nc.gpsimd.dma_start(out=wv_sb, in_=wv)
    wp_sb = consts.tile([C + 1, C + 1], BF16)
    nc.gpsimd.dma_start(out=wp_sb, in_=wp)
    pools.update(wqk_sb=wqk_sb, wv_sb=wv_sb, wp_sb=wp_sb)

    chunks = [(b, ch) for b in range(B) for ch in range(NCH)]
    preps = {0: {"xT": _load_x(nc, pools, x, 0)}}

    pT_store = {}
    yT_store = {}
    av_store = {}
    for i in range(len(chunks) + 1):
        curr = chunks[i] if i < len(chunks) else None
        prev = chunks[i - 1] if i > 0 else None

        # extra work interleaved into this pipeline stage: x DMA for the
        # next batch at its first chunk, qk/v prep ops spread over the
        # steps (batch 0's prep interleaves into its own first chunk).
        prep_ops = []
        if curr is not None:
            b, ch = curr
            if i == 0:
                prep_ops = list(_prep_qk_ops(nc, pools, preps[0])) + \
                           list(_prep_v_ops(nc, pools, preps[0]))
            if ch == 0 and b + 1 < B:
                preps[b + 1] = {"xT": _load_x(nc, pools, x, b + 1)}
            if ch == NCH - 1 and b + 1 < B:
                prep_ops = list(_prep_qk_ops(nc, pools, preps[b + 1])) + \
                           list(_prep_v_ops(nc, pools, preps[b + 1]))
            prep = preps[b]
            av = pools["ps_av"].tile([C + 1, CH], F32, tag="av")
            av_store[curr] = av
            if ch == 0:
                yT_store[b] = pools["yp"].tile([C + 1, N], F32, tag="yT",
                                               name=f"yT{b % 2}")
        if prev is not None:
            pvaug = preps[prev[0]]["vaug"]
            pav = av_store[prev]

        ops_per_step = -(-len(prep_ops) // (NB // 2))
        for u in range(NB // 2):
            # prep ops first: batch-0 scores depend on them, and the PE
            # queue is in-order
            for op in prep_ops[u * ops_per_step:(u + 1) * ops_per_step]:
                op()
            if curr is not None:
                qT, kT = prep["qT"], prep["kT"]
                for t in (2 * u, 2 * u + 1):
                    s_ps = pools["ps_s"].tile([P, CH], F32, tag="s")
                    for g in range(2):
                        nc.tensor.matmul(
                            s_ps[:, ts(g, MMF)],
                            lhsT=kT[ds(C * g, C), t, :],
                            rhs=qT[ds(C * g, C), ds(ch * CH + g * MMF, MMF)],
                            start=True, stop=True)
                    pT = pools["pTp"].tile([P, CH], BF16, tag="p",
                                           name=f"pT{t}")
                    if t in DVE_TILES:
                        nc.vector.tensor_scalar(out=pT.bitcast(I16),
                                                in0=s_ps,
                                                scalar1=float(SEXP_A),
                                                scalar2=float(SEXP_B),
                                                op0=MUL, op1=ADD)
                    else:
                        nc.scalar.activation(pT, s_ps, EXP, scale=SCALE)
                    pT_store[(curr, t)] = pT
            if prev is not None:
                for t in (2 * u, 2 * u + 1):
                    ppT = pT_store.pop((prev, t))
                    for s in range(CH // MMF):
                        nc.tensor.matmul(pav[:, ts(s, MMF)],
                                         lhsT=pvaug[:, t, :],
                                         rhs=ppT[:, ts(s, MMF)],
                                         start=(t == 0), stop=(t == NB - 1))

        if prev is not None:
            pb, pch = prev
            oT = pools["oTp"].tile([C + 1, CH], BF16, tag="oT")
            nc.vector.tensor_copy(out=oT, in_=av_store.pop(prev))
            yT_sb = yT_store[pb]
            for s in range(CH // MMF):
                psy = pools["ps_sm"].tile([C + 1, MMF], F32, tag="sm",
                                          name=f"psy{s}")
                nc.tensor.matmul(psy, lhsT=pools["wp_sb"],
                                 rhs=oT[:, ts(s, MMF)], start=True, stop=True)
                nc.vector.tensor_copy(
                    out=yT_sb[:, ds(pch * CH + s * MMF, MMF)], in_=psy)
            if pch == NCH - 1:
                nc.sync.dma_start(out=y[pb], in_=yT_store.pop(pb))


def build_kernel_nc():
    nc = bacc.Bacc("TRN2", target_bir_lowering=False, debug=False,
                   num_devices=NCORES)
    x = nc.dram_tensor("x", [B, C, N], BF16, kind="ExternalInput").ap()
    wqk = nc.dram_tensor("wqk", [P, P], BF16, kind="ExternalInput").ap()
    wv = nc.dram_tensor("wv", [P, C], BF16, kind="ExternalInput").ap()
    wp = nc.dram_tensor("wp", [C + 1, C + 1], BF16, kind="ExternalInput").ap()
    y = nc.dram_tensor("y", [B, C + 1, N], F32, kind="ExternalOutput").ap()
    with tile.TileContext(nc) as tc:
        with ExitStack() as ctx:
            _attn_kernel(ctx, tc, y, x, wqk, wv, wp)
    nc.compile()
    return nc


def make_in_maps(x, Wqkv, Wproj, bproj):
    x = np.asarray(x, dtype=np.float32)
    Wqkv = np.asarray(Wqkv, dtype=np.float32)
    Wproj = np.asarray(Wproj, dtype=np.float32)
    x_bf = np.ascontiguousarray(
        x.transpose(0, 2, 1).astype(ml_dtypes.bfloat16))

    def dup(w):  # stack for the two PE row-groups
        return np.ascontiguousarray(
            np.concatenate([w, w], axis=0).astype(ml_dtypes.bfloat16))

    in_maps = []
    for h in range(NCORES):
        wq = Wqkv[:, 0 * H * C + h * C:0 * H * C + (h + 1) * C]
        wk = Wqkv[:, 1 * H * C + h * C:1 * H * C + (h + 1) * C]
        wv = Wqkv[:, 2 * H * C + h * C:2 * H * C + (h + 1) * C]
        wqk = dup(np.concatenate([wq, wk], axis=1))
        # augmented projection: rows = 64 channels + l, cols = 64 outputs
        # + l passthrough
        wp = np.zeros((C + 1, C + 1), np.float32)
        wp[0:C, 0:C] = Wproj[h * C:(h + 1) * C, :]
        wp[C, C] = 1.0
        wp = np.ascontiguousarray(wp.astype(ml_dtypes.bfloat16))
        in_maps.append({"x": x_bf, "wqk": wqk, "wv": dup(wv), "wp": wp})
    return in_maps


_NC_CACHE = None


def _get_nc():
    global _NC_CACHE
    if _NC_CACHE is None:
        _NC_CACHE = build_kernel_nc()
    return _NC_CACHE


def run(inputs, trace=False, trace_kwargs=None):
    in_maps = make_in_maps(**inputs)
    res = run_bass_kernel_spmd(_get_nc(), in_maps, list(range(NCORES)),
                               trace=trace, **(trace_kwargs or {}))
    y = np.zeros((B, N, C), np.float32)
    for r in res.results:
        yT = r["y"].reshape(B, C + 1, N).astype(np.float32)
        y += (yT[:, 0:C, :] / yT[:, C:C + 1, :]).transpose(0, 2, 1)
    y += np.asarray(inputs["bproj"], np.float32)[None, None, :]
    return y, res


def kernel(x, Wqkv, Wproj, bproj):
    y, _ = run(dict(x=x, Wqkv=Wqkv, Wproj=Wproj, bproj=bproj))
    return y
